# revision 21
# baseline (speedup 1.0000x reference)
"""Trainium2 Bass kernel for nn_MultiHeadAttention (B=2, N=M=2048, D=1024, H=16).

Sharding: 8 cores = 2 batches x 4 head-groups (4 heads per core, tensor-parallel
over the head dim of Wq/Wk/Wv/Wp).  Each core computes a partial output
projection [N, D]; the host sums the 4 partials per batch and adds bp.

Per-core dataflow (all layouts chosen so the PE contracts over partitions;
fp16 operands for every matmul -- fp32 streams 4x slower through the PE):
  - host passes X_q^T, X_kv^T ([D, N] f16), mask^T ([M, N] f16 {0,1}) and
    per-core f16 weight slices.
  - K^T[dh, m], Q^T[dh, n] via matmul(lhsT=W chunk, rhs=X^T chunk), f32 PSUM,
    bias added in f32 by ACT during evacuation, stored f16.
  - V[m, dh] via matmul(lhsT=X^T chunk, rhs=Wv), bias via an extra ones-outer
    matmul accumulated into PSUM; stored f16 with a ones column per head.
  - S^T tile [m=128, n] = matmul(lhsT=K^T slice, rhs=Q^T); exp on ACT with
    the 1/sqrt(dh) scale folded in; mask multiply on DVE (f16, 2x mode).
  - O'^T[dh+1, n] accumulated over m-chunks: matmul(lhsT=V~[m,65], rhs=E^T);
    row 64 = softmax denominator (ones column trick).  Loop order (nh, g, m)
    so each mask half is DMA'd once.
  - normalization: rowsum rows -> PE-transpose -> DVE reciprocal ->
    PE-transpose back -> broadcast over dh via a select-row matmul ->
    in-place DVE multiply on O^T.
  - out[t, D] = sum_h matmul(lhsT=O^T_h slice, rhs=Wp_h), PSUM-accumulated.
"""

import numpy as np
from contextlib import ExitStack

import concourse.bass as bass
import concourse.tile as tile
from concourse import mybir
from concourse.bass_utils import run_bass_kernel_spmd
from concourse.vector_clock import ScopedClock
from concourse.masks import make_identity

B, N, M, D = 2, 2048, 2048, 1024
H = 16
DH = D // H  # 64
SCALE = DH ** -0.5
NCORES = 8
HG = 4            # heads per core
CSL = HG * DH     # 256 columns of Wq/Wk/Wv per core
F32 = mybir.dt.float32
F16 = mybir.dt.float16

# ---------------------------------------------------------------------------
# walrus in this container rejects >1 sem wait per instruction; spread the
# extras across preceding same-engine NOPs (queues execute in order, so this
# is semantically identical).
_MAX_WAITS = 1


def _patched_drain_and_barrier(self, tick_clock, wait_clock):
    drain_inst = self.nc.sync.drain()
    wait_clock.add_sem_waits(
        drain_inst.ins, ScopedClock({None: tick_clock.global_clock})
    )
    si = drain_inst.ins.sync_info
    waits = list(si.on_wait or []) if si else []
    if len(waits) > _MAX_WAITS:
        si.on_wait = waits[:_MAX_WAITS]
        for i in range(_MAX_WAITS, len(waits), _MAX_WAITS):
            extra = self.nc.sync.drain()
            extra.ins.sync_info = mybir.SyncInfo(
                on_wait=waits[i : i + _MAX_WAITS], on_update=[]
            )
    self.nc.all_engine_barrier()
    assert self.sems is not None
    popped = self.nc._tile_sem_poison_stack.pop()
    assert popped is self._sem_poison
    self.nc.clear_and_free_semaphores(list(self.sems.allocated().values()))
    self.nc.all_engine_barrier()


tile.TileContext._drain_and_barrier = _patched_drain_and_barrier
# ---------------------------------------------------------------------------

Exp = mybir.ActivationFunctionType.Exp
Identity = mybir.ActivationFunctionType.Identity


def _split_waits(nc):
    n_split = 0
    for bb in nc.main_func.blocks:
        new_list = []
        for ins in bb.instructions:
            si = ins.sync_info
            if si is not None and si.on_wait and len(si.on_wait) > 1:
                waits = list(si.on_wait)
                for j, w in enumerate(waits[:-1]):
                    nop = mybir.InstNoOp(
                        name=f"{ins.name}-sw{j}",
                        engine=ins.engine,
                        sync_info=mybir.SyncInfo(on_wait=[w], on_update=[]),
                    )
                    new_list.append(nop)
                    n_split += 1
                si.on_wait = [waits[-1]]
            new_list.append(ins)
        bb.instructions = new_list
    return n_split


def build_nc(reps: int = 1) -> bass.Bass:
    nc = bass.Bass()

    xqT = nc.dram_tensor("xqT", [D, N], F16, kind="ExternalInput")
    xkvT = nc.dram_tensor("xkvT", [D, M], F16, kind="ExternalInput")
    maskT = nc.dram_tensor("maskT", [M, N], F16, kind="ExternalInput")
    wq = nc.dram_tensor("wq", [D, CSL], F16, kind="ExternalInput")
    wk = nc.dram_tensor("wk", [D, CSL], F16, kind="ExternalInput")
    wv = nc.dram_tensor("wv", [D, CSL], F16, kind="ExternalInput")
    wp = nc.dram_tensor("wp", [CSL, D], F16, kind="ExternalInput")
    bq2 = nc.dram_tensor("bq2", [128, 2], F32, kind="ExternalInput")
    bk2 = nc.dram_tensor("bk2", [128, 2], F32, kind="ExternalInput")
    bv1 = nc.dram_tensor("bv1", [1, CSL], F16, kind="ExternalInput")
    sel4in = nc.dram_tensor("sel4in", [4, HG * DH], F16, kind="ExternalInput")
    outp = nc.dram_tensor("outp", [N, D], F16, kind="ExternalOutput")
    rscratch = nc.dram_tensor("rscratch", [4, N], F16)

    MT = M // 128   # 16 m-chunks
    NT = N // 128   # 16 t-tiles

    with ExitStack() as ctx:
        tc = ctx.enter_context(tile.TileContext(nc))

        consts = ctx.enter_context(tc.tile_pool(name="consts", bufs=1))
        ident = consts.tile([128, 128], F16)
        make_identity(nc, ident)
        ones_row = consts.tile([1, 128], F16)
        nc.vector.memset(ones_row, 1.0)
        sel4 = consts.tile([4, 4, DH], F16)
        nc.sync.dma_start(out=sel4, in_=sel4in[:, :])
        bq_sb = consts.tile([128, 2], F32)
        nc.sync.dma_start(out=bq_sb, in_=bq2[:, :])
        bk_sb = consts.tile([128, 2], F32)
        nc.sync.dma_start(out=bk_sb, in_=bk2[:, :])
        bv_sb = consts.tile([1, CSL], F16)
        nc.sync.dma_start(out=bv_sb, in_=bv1[:, :])
        wp_sb = consts.tile([128, 2, D], F16)
        for g in range(2):
            nc.sync.dma_start(out=wp_sb[:, g, :], in_=wp[g * 128 : (g + 1) * 128, :])

        persist = ctx.enter_context(tc.tile_pool(name="persist", bufs=1))
        KT = [persist.tile([128, M], F16, tag=f"kt{g}", name=f"kt{g}") for g in range(2)]
        QT = [persist.tile([128, N], F16, tag=f"qt{g}", name=f"qt{g}") for g in range(2)]
        V = persist.tile([128, MT, HG, DH + 1], F16, tag="v")
        OTALL = persist.tile([DH + 1, HG, N], F16, tag="ot")
        OT = [OTALL[:, h, :] for h in range(HG)]
        OTP = [persist.tile([128, N], F16, tag=f"otp{g}", name=f"otp{g}") for g in range(2)]
        rowsums = persist.tile([4, N], F16, tag="rs")
        recrows = persist.tile([4, N], F16, tag="rr")
        recipT = persist.tile([128, 64], F16, tag="rcpt")

        # mask halves ([128, MT, 1024] f16 = 32KB/partition each)
        maskp = ctx.enter_context(tc.tile_pool(name="maskp", bufs=1))

        def _rep_body():
            # ------- staging + projections (KT0,QT0,KT1,QT1 first; V last,
            # it is off the critical path to the first attention block) ------
            with (
                tc.tile_pool(name="xkv", bufs=1) as xkvp,
                tc.tile_pool(name="wkv", bufs=1) as wkvp,
                tc.tile_pool(name="xq", bufs=1) as xqp,
                tc.tile_pool(name="wqp", bufs=1) as wqpool,
                tc.tile_pool(name="pp1", bufs=2, space="PSUM") as pp1,
            ):
                xkv_sb = xkvp.tile([128, 8, M], F16)
                for d in range(8):
                    nc.sync.dma_start(
                        out=xkv_sb[:, d, :], in_=xkvT[d * 128 : (d + 1) * 128, :]
                    )
                wk_sb = wkvp.tile([128, 8, CSL], F16)
                wv_sb = wkvp.tile([128, 8, CSL], F16)
                for d in range(8):
                    nc.sync.dma_start(out=wk_sb[:, d, :], in_=wk[d * 128 : (d + 1) * 128, :])
                    nc.sync.dma_start(out=wv_sb[:, d, :], in_=wv[d * 128 : (d + 1) * 128, :])
                xq_sb = xqp.tile([128, 8, N], F16)
                for d in range(8):
                    nc.sync.dma_start(
                        out=xq_sb[:, d, :], in_=xqT[d * 128 : (d + 1) * 128, :]
                    )
                wq_sb = wqpool.tile([128, 8, CSL], F16)
                for d in range(8):
                    nc.sync.dma_start(out=wq_sb[:, d, :], in_=wq[d * 128 : (d + 1) * 128, :])
                mk_halves = [maskp.tile([128, MT, 1024], F16, tag="mk", name="mk0")]
                for m in range(MT):
                    nc.sync.dma_start(
                        out=mk_halves[0][:, m, :],
                        in_=maskT[m * 128 : (m + 1) * 128, 0:1024],
                    )

                def project(g, w_sb, dst, bias_sb):
                    for ms in range(4):
                        ps = pp1.tile([128, 512], F32, tag="proj", name="ps")
                        for d in range(8):
                            nc.tensor.matmul(
                                ps,
                                w_sb[:, d, g * 128 : (g + 1) * 128],
                                (xkv_sb if w_sb is wk_sb else xq_sb)[
                                    :, d, ms * 512 : (ms + 1) * 512
                                ],
                                start=(d == 0),
                                stop=(d == 7),
                            )
                        # bias-add on DVE: keeps the ACT queue clear so the
                        # first attention exp isn't FIFO-blocked behind these
                        nc.vector.tensor_scalar_add(
                            dst[:, ms * 512 : (ms + 1) * 512],
                            ps,
                            bias_sb[:, g : g + 1],
                        )

                project(0, wk_sb, KT[0], bk_sb)
                project(0, wq_sb, QT[0], bq_sb)

                for mt in range(MT):
                    vp = pp1.tile([128, HG, DH], F32, tag="vproj")
                    for d in range(8):
                        nc.tensor.matmul(
                            vp,
                            xkv_sb[:, d, mt * 128 : (mt + 1) * 128],
                            wv_sb[:, d, :],
                            start=(d == 0),
                            stop=False,
                        )
                    nc.tensor.matmul(vp, ones_row, bv_sb, start=False, stop=True)
                    nc.vector.tensor_copy(V[:, mt, :, 0:DH], vp)
                nc.vector.memset(V[:, :, :, DH : DH + 1], 1.0)

                project(1, wk_sb, KT[1], bk_sb)
                project(1, wq_sb, QT[1], bq_sb)

            # ---------------- phase 3: attention ----------------------------
            with (
                tc.tile_pool(name="et", bufs=6) as etp,
                tc.tile_pool(name="otp", bufs=1, space="PSUM") as otpp,
                tc.tile_pool(name="stp", bufs=1, space="PSUM") as stp,
            ):
                for nh in range(2):
                    nhs = nh * 1024
                    if nh == 1:
                        mk = maskp.tile([128, MT, 1024], F16, tag="mk", name="mk1")
                        for m in range(MT):
                            nc.sync.dma_start(
                                out=mk[:, m, :],
                                in_=maskT[m * 128 : (m + 1) * 128, 1024:2048],
                            )
                    else:
                        mk = mk_halves[0]
                    for g in range(2):
                        ota = otpp.tile([DH + 1, 1024], F32, tag="ota")
                        otb = otpp.tile([DH + 1, 1024], F32, tag="otb")
                        for m in range(MT):
                            sta = stp.tile([128, 1024], F32, tag="sta")
                            stb = stp.tile([128, 1024], F32, tag="stb")
                            for ns in range(2):
                                nc.tensor.matmul(
                                    sta[:, ns * 512 : (ns + 1) * 512],
                                    KT[g][0:64, m * 128 : (m + 1) * 128],
                                    QT[g][0:64, nhs + ns * 512 : nhs + (ns + 1) * 512],
                                )
                                nc.tensor.matmul(
                                    stb[:, ns * 512 : (ns + 1) * 512],
                                    KT[g][64:128, m * 128 : (m + 1) * 128],
                                    QT[g][64:128, nhs + ns * 512 : nhs + (ns + 1) * 512],
                                )
                            eta = etp.tile([128, 1024], F16, tag="eta")
                            etb = etp.tile([128, 1024], F16, tag="etb")
                            nc.scalar.activation(eta, sta, Exp, scale=SCALE)
                            nc.scalar.activation(etb, stb, Exp, scale=SCALE)
                            mule = nc.gpsimd.tensor_mul if m < 2 else nc.vector.tensor_mul
                            mule(eta, eta, mk[:, m, :])
                            mule(etb, etb, mk[:, m, :])
                            for ns in range(2):
                                nc.tensor.matmul(
                                    ota[:, ns * 512 : (ns + 1) * 512],
                                    V[:, m, 2 * g, :],
                                    eta[:, ns * 512 : (ns + 1) * 512],
                                    start=(m == 0),
                                    stop=(m == MT - 1),
                                )
                                nc.tensor.matmul(
                                    otb[:, ns * 512 : (ns + 1) * 512],
                                    V[:, m, 2 * g + 1, :],
                                    etb[:, ns * 512 : (ns + 1) * 512],
                                    start=(m == 0),
                                    stop=(m == MT - 1),
                                )
                        nc.vector.tensor_copy(OT[2 * g][:, nhs : nhs + 1024], ota)
                        nc.vector.tensor_copy(OT[2 * g + 1][:, nhs : nhs + 1024], otb)

            # rowsum rows -> one [4, N] tile (partition shift => DMA)
            nc.sync.dma_start(
                out=rowsums, in_=OTALL[DH : DH + 1, :, :]
            )

            # ---------------- phase 4: normalize ----------------------------
            with tc.tile_pool(name="np1", bufs=1, space="PSUM") as np1:
                rsT = np1.tile([128, 64], F16, tag="rst")
                for b in range(16):
                    nc.tensor.transpose(
                        rsT[:, 4 * b : 4 * b + 4],
                        rowsums[:, b * 128 : (b + 1) * 128],
                        ident[0:4, 0:4],
                    )
                with nc.allow_low_precision(
                    reason="softmax denominators are O(1e3); f16 recip gives "
                    "~5e-4 rel err, well inside the output tolerance"
                ):
                    nc.vector.reciprocal(recipT, rsT)
                rrps = np1.tile([4, N], F16, tag="rrps")
                for b in range(16):
                    nc.tensor.transpose(
                        rrps[:, b * 128 : (b + 1) * 128],
                        recipT[:, 4 * b : 4 * b + 4],
                        ident,
                    )
                nc.vector.tensor_copy(recrows, rrps)

            with tc.tile_pool(name="np2", bufs=2, space="PSUM") as np2:
                for h in range(HG):
                    g, sub = divmod(h, 2)
                    rps = np2.tile([DH, N], F32, tag="rbc")
                    for ns in range(4):
                        nc.tensor.matmul(
                            rps[:, ns * 512 : (ns + 1) * 512],
                            sel4[:, h, :],
                            recrows[:, ns * 512 : (ns + 1) * 512],
                        )
                    # multiply straight from the PSUM broadcast rows (DVE
                    # accepts the f16 x f32-PSUM mixed operands), skipping
                    # the 16 serial evacuation copies
                    if sub == 0:
                        nc.vector.tensor_mul(OTP[g][0:DH, :], OT[h][0:DH, :], rps)
                    else:
                        nc.vector.tensor_mul(OT[h][0:DH, :], OT[h][0:DH, :], rps)
                        # partition shift 0-63 -> 64-127 (only DMA can do this)
                        nc.sync.dma_start(
                            out=OTP[g][DH:128, :], in_=OT[h][0:DH, :]
                        )

            # ---------------- phase 5: output projection --------------------
            with (
                tc.tile_pool(name="ops", bufs=4, space="PSUM") as opp,
                tc.tile_pool(name="osb", bufs=6) as osb,
            ):
                for t in range(NT):
                    po = opp.tile([128, D], F32, tag="po")
                    for g in range(2):
                        for ns in range(2):
                            nc.tensor.matmul(
                                po[:, ns * 512 : (ns + 1) * 512],
                                OTP[g][:, t * 128 : (t + 1) * 128],
                                wp_sb[:, g, ns * 512 : (ns + 1) * 512],
                                start=(g == 0),
                                stop=(g == 1),
                            )
                    ob = osb.tile([128, D], F16, tag="ob")
                    if t % 2 == 0:
                        nc.scalar.copy(ob, po)
                    else:
                        nc.vector.tensor_copy(ob, po)
                    nc.sync.dma_start(out=outp[t * 128 : (t + 1) * 128, :], in_=ob)

        if reps == 1:
            _rep_body()
        else:
            # hardware loop keeps program size (and compile time) constant
            # regardless of the benchmark repeat count
            with tc.For_i(0, reps):
                _rep_body()

    _split_waits(nc)
    return nc


_SEL4 = np.zeros((4, 4, DH), dtype=np.float16)
for _h in range(4):
    _SEL4[_h, _h, :] = 1.0
_SEL4 = np.ascontiguousarray(_SEL4.reshape(4, 4 * DH))

_NC_CACHE = {}
_TRACE = False
_LAST_EXEC_NS = None


def _get_nc():
    if "nc" not in _NC_CACHE:
        _NC_CACHE["nc"] = build_nc()
    return _NC_CACHE["nc"]


def kernel(
    inputs_kv, inputs_q, attention_mask, Wq, bq, Wk, bk, Wv, bv, Wp, bp, **_unused
):
    inputs_kv = np.asarray(inputs_kv, dtype=np.float32)
    inputs_q = np.asarray(inputs_q, dtype=np.float32)
    attention_mask = np.asarray(attention_mask)
    Wq = np.asarray(Wq, dtype=np.float32)
    Wk = np.asarray(Wk, dtype=np.float32)
    Wv = np.asarray(Wv, dtype=np.float32)
    Wp = np.asarray(Wp, dtype=np.float32)
    bq = np.asarray(bq, dtype=np.float32)
    bk = np.asarray(bk, dtype=np.float32)
    bv = np.asarray(bv, dtype=np.float32)
    bp = np.asarray(bp, dtype=np.float32)

    in_maps = []
    for c in range(NCORES):
        bidx, g = divmod(c, HG)
        cs = slice(g * CSL, (g + 1) * CSL)
        in_maps.append(
            {
                "xqT": np.ascontiguousarray(inputs_q[bidx].T.astype(np.float16)),
                "xkvT": np.ascontiguousarray(inputs_kv[bidx].T.astype(np.float16)),
                "maskT": np.ascontiguousarray(
                    attention_mask[bidx, 0].T.astype(np.float16)
                ),
                "wq": np.ascontiguousarray(Wq[:, cs].astype(np.float16)),
                "wk": np.ascontiguousarray(Wk[:, cs].astype(np.float16)),
                "wv": np.ascontiguousarray(Wv[:, cs].astype(np.float16)),
                "wp": np.ascontiguousarray(Wp[cs, :].astype(np.float16)),
                "bq2": np.ascontiguousarray(bq[cs].reshape(2, 128).T),
                "bk2": np.ascontiguousarray(bk[cs].reshape(2, 128).T),
                "bv1": np.ascontiguousarray(bv[cs].reshape(1, CSL).astype(np.float16)),
                "sel4in": _SEL4,
            }
        )

    nc = _get_nc()
    res = run_bass_kernel_spmd(
        nc, in_maps, core_ids=list(range(NCORES)), trace=_TRACE
    )
    global _LAST_EXEC_NS
    _LAST_EXEC_NS = res.exec_time_ns

    out = np.zeros((B, N, D), dtype=np.float32)
    for c in range(NCORES):
        bidx = c // HG
        out[bidx] += res.results[c]["outp"].astype(np.float32)
    out += bp
    return out

